# revision 37
# baseline (speedup 1.0000x reference)
"""DiT attention kernel for 8 Trainium2 NeuronCores — pipelined v3.

Sharding: tensor-parallel over head groups (4 groups of 4 heads) x
data-parallel over batch (2), giving 8 cores. Each core computes, for its
batch b and head group g:
    QT/KT = (x[b] @ W{q,k}[g].T + b)  in [head_dim, seq] layout
    partial rotary on global head 0 (cores with g==0; others get cos=1/sin=0)
    V in [seq, head_dim] layout with a per-head ones column (so the PV
    matmul also produces softmax denominators), S^T = K Q^T per head,
    P^T = exp(S^T/8) in bf16, O^T = V_aug^T P^T, normalize via Pool-engine
    partition broadcast of the reciprocal denominators, then the
    row-parallel Wo partial product out^T = Wo[g].T @ O^T.
The host sums the 4 partial out^T per batch, transposes, and adds bo.

v3: single software-pipelined emission.  The scalar engine's exp stream
(134us, the phase-2 bottleneck) starts ~10us in and overlaps all PE work:
x streams in bf16 column pieces (first projection starts at ~4us), each
score sweep carries its own PV matmuls two blocks behind, projections /
V / rope / output-projection blocks fill every PE bubble, rotate_half is
a PE permutation matmul (no DMA), softmax normalization broadcasts
reciprocal denominators with gpsimd (no DRAM round-trips), and the final
output quarter borrows the score/PV psum banks so its Wo matmuls overlap
the last normalization chain.
"""

import sys

if "/opt/trn_rl_repo" not in sys.path:
    sys.path.insert(0, "/opt/trn_rl_repo")

from collections import deque
from contextlib import ExitStack

import numpy as np

import concourse.bass as bass  # noqa: F401  (bass must import before bacc)
import concourse.mybir as mybir
import concourse.tile as tile
from concourse import bacc
from concourse.bass_utils import run_bass_kernel_spmd

F32 = mybir.dt.float32
F32R = mybir.dt.float32r
BF16 = mybir.dt.bfloat16

B, S, DIM, HEADS, HEAD_DIM = 2, 2048, 1024, 16, 64
N_CORES = 8
TP = 4                      # head groups
GH = HEADS // TP            # heads per core (4)
GC = GH * HEAD_DIM          # cols per core slice (256)
EXP_FN = mybir.ActivationFunctionType.Exp


def _emit_body(nc, tc, ctx, d):
    consts = ctx.enter_context(tc.tile_pool(name="consts", bufs=1))

    # ---------------- persistent tiles ------------------------------------
    qt = [consts.tile([128, S], F32R, name=f"qt{m}", tag=f"qt{m}") for m in range(2)]
    kt = [consts.tile([128, S], F32R, name=f"kt{m}", tag=f"kt{m}") for m in range(2)]
    # V in natural layout + ones column per head: head h at cols [65h, 65h+64),
    # col 65h+64 is 1.0 (gives softmax denominators as row 64 of the PV psum).
    vst = [consts.tile([128, 65 * GH], BF16, name=f"vs{i}", tag=f"vs{i}") for i in range(16)]
    otst = [
        [
            consts.tile([128, 512], F32R, name=f"ot{m}{n}", tag=f"ot{m}{n}")
            for n in range(4)
        ]
        for m in range(2)
    ]
    # x^T as one [128, 8*2048] bf16 tile: chunk k (x dims 128k:128k+128) at
    # cols [2048k, 2048k+2048); loaded in four 512-wide column pieces so the
    # first projection quarter only waits for piece 0.
    xtb = consts.tile([128, 8 * S], BF16, name="xtb", tag="xtb")
    wqb = consts.tile([128, 8 * GC], BF16, name="wqb", tag="wqb")
    wkb = consts.tile([128, 8 * GC], BF16, name="wkb", tag="wkb")
    wvb = consts.tile([128, 8 * GC], BF16, name="wvb", tag="wvb")
    wob = consts.tile([128, 2 * DIM], F32R, name="wob", tag="wob")
    cos_sb = consts.tile([64, S], F32R, name="cos", tag="cos")
    sin_sb = consts.tile([64, S], F32R, name="sin", tag="sin")
    bq_sb = consts.tile([128, 2], F32, name="bq", tag="bq")
    bk_sb = consts.tile([128, 2], F32, name="bk", tag="bk")
    bvrep = consts.tile([128, GC], F32, name="bvrep", tag="bvrep")
    perm_sb = consts.tile([64, 64], F32R, name="perm", tag="perm")
    ones1 = consts.tile([128, 64], F32R, name="ones1", tag="ones1")
    warm = consts.tile([1, 4], F32, name="warm", tag="warm")

    def xt(k):
        return xtb[:, S * k : S * (k + 1)]

    # ACT warmup: load the Exp table before any real dependency exists.
    nc.vector.memset(warm[:], 0.0)
    nc.scalar.activation(warm[:], warm[:], EXP_FN, scale=1.0)

    # ---------------- input DMAs (SP HWDGE ring, FIFO) --------------------
    # Plain partition-first slice DMAs (SBUF APs with the partition dim not
    # first do not lower correctly); ordered so K0-quarter-0's data lands
    # first: wk + the first 512-column piece of each x chunk.
    def load_w(dstb, src, k):
        nc.sync.dma_start(
            out=dstb[:, GC * k : GC * (k + 1)], in_=src[128 * k : 128 * (k + 1), :]
        )

    def load_x(k, j):
        nc.sync.dma_start(
            out=xtb[:, S * k + 512 * j : S * k + 512 * (j + 1)],
            in_=d["xT"][128 * k : 128 * (k + 1), 512 * j : 512 * (j + 1)],
        )

    for k in range(8):
        load_w(wkb, d["wk"], k)
        load_x(k, 0)
    nc.sync.dma_start(out=perm_sb[:], in_=d["perm64"][:, :])
    nc.sync.dma_start(out=ones1[64:65, :], in_=d["ones64"][:, :])
    nc.sync.dma_start(out=bk_sb[:], in_=d["bk2"][:, :])
    nc.sync.dma_start(out=bq_sb[:], in_=d["bq2"][:, :])
    nc.sync.dma_start(out=cos_sb[:], in_=d["cosT"][:, :])
    nc.sync.dma_start(out=sin_sb[:], in_=d["sinT"][:, :])
    for k in range(8):
        load_w(wqb, d["wq"], k)
        load_x(k, 1)
    for k in range(8):
        load_w(wvb, d["wv"], k)
        load_x(k, 2)
    nc.sync.dma_start(out=bvrep[:], in_=d["bvrow"][:].to_broadcast([128, GC]))
    for k in range(8):
        load_x(k, 3)
    for k in range(2):
        nc.sync.dma_start(
            out=wob[:, DIM * k : DIM * (k + 1)], in_=d["wo"][128 * k : 128 * (k + 1), :]
        )

    # ---------------- pools (PSUM: 2 + 4 + 2 = 8 banks) --------------------
    pa = ctx.enter_context(tc.tile_pool(name="pa", bufs=2, space="PSUM"))    # [128,512]
    pst = ctx.enter_context(tc.tile_pool(name="pst", bufs=2, space="PSUM"))  # [128,1024]
    pot = ctx.enter_context(tc.tile_pool(name="pot", bufs=2, space="PSUM"))  # [128,512]
    ptp = ctx.enter_context(tc.tile_pool(name="ptp", bufs=8))                # pt bf16
    rpp = ctx.enter_context(tc.tile_pool(name="rpp", bufs=2))                # rope tmp
    nrm = ctx.enter_context(tc.tile_pool(name="nrm", bufs=2))                # rst/bc
    obp = ctx.enter_context(tc.tile_pool(name="obp", bufs=4))                # out stage

    # ---------------- emission helpers ------------------------------------
    def proj_quarter(dst, w_sb, bias_sb, m, n):
        ps = pa.tile([128, 512], F32, name="pa", tag="pa")
        for k in range(8):
            nc.tensor.matmul(
                ps[:],
                lhsT=w_sb[:, GC * k + 128 * m : GC * k + 128 * (m + 1)],
                rhs=xt(k)[:, 512 * n : 512 * (n + 1)],
                start=(k == 0),
                stop=(k == 7),
            )
        nc.vector.tensor_scalar_add(
            out=dst[:, 512 * n : 512 * (n + 1)], in0=ps[:], scalar1=bias_sb[:, m : m + 1]
        )

    def rope_quarter(dstt, n):
        # rotate_half via a PE permutation matmul (no DMA): psum gets the
        # pair-swapped rows, sin_sb already carries the (-1,+1) signs.
        sl = slice(512 * n, 512 * (n + 1))
        ps = pa.tile([128, 512], F32, name="pa", tag="pa")
        nc.tensor.matmul(
            ps[0:64, :], lhsT=perm_sb[:, :], rhs=dstt[0:64, sl], start=True, stop=True
        )
        t1 = rpp.tile([64, 512], F32R, name="t1", tag="t1", bufs=2)
        nc.vector.tensor_mul(t1[:], ps[0:64, :].bitcast(F32R), sin_sb[:, sl])
        nc.vector.tensor_mul(dstt[0:64, sl], dstt[0:64, sl], cos_sb[:, sl])
        nc.vector.tensor_add(dstt[0:64, sl], dstt[0:64, sl], t1[:])

    def v_chunk(sc):
        ps = pa.tile([128, 512], F32, name="pa", tag="pa")
        for k in range(8):
            nc.tensor.matmul(
                ps[:, 0:GC],
                lhsT=xt(k)[:, 128 * sc : 128 * (sc + 1)],
                rhs=wvb[:, GC * k : GC * (k + 1)],
                start=(k == 0),
                stop=(k == 7),
            )
        pv3 = ps[:, 0:GC].rearrange("p (h c) -> p h c", h=GH)
        bv3 = bvrep[:].rearrange("p (h c) -> p h c", h=GH)
        vt3 = vst[sc][:, 0 : 65 * GH].rearrange("p (h c) -> p h c", h=GH, c=65)
        nc.vector.tensor_add(vt3[:, :, 0:64], pv3[:], bv3[:])

    def scores_blk(p, qq, blk):
        st = pst.tile([128, 1024], F32, name="st", tag="st")
        for hh in range(2):
            nc.tensor.matmul(
                st[:, 512 * hh : 512 * (hh + 1)],
                lhsT=kt[p][64 * hh : 64 * (hh + 1), 128 * blk : 128 * (blk + 1)],
                rhs=qt[p][64 * hh : 64 * (hh + 1), 512 * qq : 512 * (qq + 1)],
                start=True,
                stop=True,
            )
        pt = ptp.tile([128, 1024], BF16, name="pt", tag="pt", bufs=8)
        nc.scalar.activation(pt[:], st[:], EXP_FN, scale=0.125)
        return pt

    def pv_blk(p, pts, blk, pots):
        for hh in range(2):
            h = 2 * p + hh
            nc.tensor.matmul(
                pots[hh][0:65, :],
                lhsT=vst[blk][:, 65 * h : 65 * h + 65],
                rhs=pts[blk][:, 512 * hh : 512 * (hh + 1)],
                start=(blk == 0),
                stop=(blk == 15),
            )

    def norm(p, qq, pots):
        # Denominators are psum row 64 of each pot. Evict the whole [65,512]
        # pot to SBUF (base-0 DVE copy), reciprocal rows 0:65 (row 64 is the
        # one that matters), broadcast row 64 across 64 partitions with a K=1
        # PE matmul (lhsT and rhs both at base 64), then one DVE mul.
        ot = otst[p][qq]
        for hh in (1, 0):
            otun = nrm.tile([128, 512], F32, name="otun", tag="otun", bufs=2)
            rst = nrm.tile([128, 512], F32, name="rst", tag="rst", bufs=2)
            bc = nrm.tile([128, 512], F32, name="bc", tag="bc", bufs=2)
            nc.vector.tensor_copy(otun[0:65, :], pots[hh][0:65, :])
            with nc.allow_low_precision(reason="f32r bits == f32 bits"):
                nc.vector.reciprocal(rst[0:65, :].bitcast(F32R), otun[0:65, :])
            bcp = pa.tile([128, 512], F32, name="pa", tag="pa")
            nc.tensor.matmul(
                bcp[0:64, :],
                lhsT=ones1[64:65, :],
                rhs=rst[64:65, :].bitcast(F32R),
                start=True,
                stop=True,
            )
            nc.vector.tensor_copy(bc[0:64, :], bcp[0:64, :])
            if hh == 0:
                nc.vector.tensor_mul(
                    ot[0:64, :], otun[0:64, :], bc[0:64, :].bitcast(F32R)
                )
            else:
                # DVE cannot write partition base 64 from base-0 inputs; go
                # through a temp tile + sbuf->sbuf DMA.
                tmp = nrm.tile([64, 512], F32R, name="tmp", tag="tmp", bufs=2)
                nc.vector.tensor_mul(tmp[:], otun[0:64, :], bc[0:64, :].bitcast(F32R))
                nc.sync.dma_start(out=ot[64:128, :], in_=tmp[:])

    def wo_mm(ps, m, n, k, start, stop):
        nc.tensor.matmul(
            ps,
            lhsT=wob[:, DIM * k + 128 * m : DIM * k + 128 * (m + 1)],
            rhs=otst[k][n][:, :],
            start=start,
            stop=stop,
        )

    def ph3_m(n, m, ob, j):
        ps = pa.tile([128, 512], F32, name="pa", tag="pa")
        wo_mm(ps[:], m, n, 0, True, False)
        wo_mm(ps[:], m, n, 1, False, True)
        nc.vector.tensor_copy(ob[:, 512 * j : 512 * (j + 1)], ps[:])

    def ph3_dma(n, pair, ob):
        for j in range(2):
            nc.sync.dma_start(
                out=d["outT4"][n, 2 * pair + j], in_=ob[:, 512 * j : 512 * (j + 1)]
            )

    def ph3_dma1(n, m, ob, j):
        nc.sync.dma_start(
            out=d["outT4"][n, m], in_=ob[:, 512 * j : 512 * (j + 1)]
        )

    def ph3_last(n):
        # final output quarter: scores/PV psum banks are free now, so all 8
        # m-chunks get their own psum slot; the k=0 matmuls (which only need
        # head-pair 0, normalized a sweep earlier) overlap the tail of the
        # last normalization chain.
        slots = [pa.tile([128, 512], F32, name="pa", tag="pa") for _ in range(2)]
        for _ in range(2):
            big = pst.tile([128, 1024], F32, name="st", tag="st")
            slots += [big[:, 0:512], big[:, 512:1024]]
        slots += [pot.tile([128, 512], F32, name="pot", tag="pot") for _ in range(2)]
        for m in range(8):
            wo_mm(slots[m], m, n, 0, True, False)
        for m in range(8):
            wo_mm(slots[m], m, n, 1, False, True)
        for pair in range(4):
            ob = obp.tile([128, 1024], BF16, name="ob", tag="ob", bufs=4)
            for j in range(2):
                nc.vector.tensor_copy(
                    ob[:, 512 * j : 512 * (j + 1)], slots[2 * pair + j]
                )
                ph3_dma1(n, 2 * pair + j, ob, j)

    # ---------------- pipelined emission -----------------------------------
    fillers = deque()

    def emit_fillers(k):
        while k > 0 and fillers:
            fillers.popleft()()
            k -= 1

    done_quarters = set()

    def push_ph3(n):
        for pair in range(4):
            ob = obp.tile([128, 1024], BF16, name="ob", tag="ob", bufs=4)
            for j in range(2):
                fillers.append(
                    lambda m=2 * pair + j, j=j, ob=ob: ph3_m(n, m, ob, j)
                )
            fillers.append(lambda pair=pair, ob=ob: ph3_dma(n, pair, ob))

    def finish_quarter(p, qq, pots):
        norm(p, qq, pots)
        done_quarters.add((p, qq))
        if (0, qq) in done_quarters and (1, qq) in done_quarters:
            if qq == 3:
                ph3_last(qq)
            else:
                push_ph3(qq)

    # ones columns of V (DVE is idle early; vst writes precede v_chunk adds)
    for i in range(16):
        nc.vector.memset(vst[i][:, 64 : 65 * GH : 65], 1.0)

    # minimal prologue: K quarter 0 + rope, Q quarter 0 + rope; everything
    # else becomes sweep fillers.
    proj_quarter(kt[0], wkb, bk_sb, 0, 0)
    rope_quarter(kt[0], 0)
    proj_quarter(qt[0], wqb, bq_sb, 0, 0)
    rope_quarter(qt[0], 0)

    def f_projrope(dst, w_sb, b_sb, m, n, roped):
        def f():
            proj_quarter(dst, w_sb, b_sb, m, n)
            if roped:
                rope_quarter(dst, n)
        return f

    for n in range(1, 4):
        fillers.append(f_projrope(kt[0], wkb, bk_sb, 0, n, True))
    for n in range(4):
        fillers.append(f_projrope(kt[1], wkb, bk_sb, 1, n, False))
    fillers.append(f_projrope(qt[1], wqb, bq_sb, 1, 0, False))
    for sc in range(8):
        fillers.append(lambda sc=sc: v_chunk(sc))
    for sc in range(8, 16):
        fillers.append(lambda sc=sc: v_chunk(sc))
    for n in range(1, 4):
        fillers.append(f_projrope(qt[1], wqb, bq_sb, 1, n, False))
    for n in range(1, 4):
        fillers.append(f_projrope(qt[0], wqb, bq_sb, 0, n, True))

    # quarter-pairs adjacent: ph3(n) can fire right after sweep (1, n).
    sweep_order = [(0, 0), (1, 0), (0, 1), (1, 1), (0, 2), (1, 2), (0, 3), (1, 3)]

    for si, (p, qq) in enumerate(sweep_order):
        pts = []
        if si == 0:
            # sweep 0: scores interleaved with projection/V fillers; its PV
            # runs as a block right after (vst isn't complete mid-sweep yet).
            for blk in range(16):
                emit_fillers(1)
                pts.append(scores_blk(p, qq, blk))
            emit_fillers(8)
            pots = [pot.tile([128, 512], F32, name="pot", tag="pot") for _ in range(2)]
            for blk in range(16):
                pv_blk(p, pts, blk, pots)
            finish_quarter(p, qq, pots)
        else:
            # steady state: this sweep's PV trails its scores by 2 blocks, so
            # the PV stream drains the exp pipeline with no cross-sweep lag.
            pots = [pot.tile([128, 512], F32, name="pot", tag="pot") for _ in range(2)]
            for blk in range(16):
                if blk < 2 or (si < 7 and blk % 3 == 2):
                    emit_fillers(1)
                pts.append(scores_blk(p, qq, blk))
                if blk >= 2:
                    pv_blk(p, pts, blk - 2, pots)
            pv_blk(p, pts, 14, pots)
            pv_blk(p, pts, 15, pots)
            finish_quarter(p, qq, pots)
            emit_fillers(len(fillers) if si == 6 else 6)

    emit_fillers(len(fillers) + 4)


def build_nc(reps: int = 1, phases=(1, 2, 3)):
    nc = bacc.Bacc(
        "TRN2", target_bir_lowering=False, debug=False, num_devices=N_CORES
    )
    d = {}
    d["xT"] = nc.dram_tensor("xT", [DIM, S], BF16, kind="ExternalInput").ap()
    for nm in ("wq", "wk", "wv"):
        d[nm] = nc.dram_tensor(nm, [DIM, GC], BF16, kind="ExternalInput").ap()
    for nm in ("bq2", "bk2"):
        d[nm] = nc.dram_tensor(nm, [128, 2], F32, kind="ExternalInput").ap()
    d["bvrow"] = nc.dram_tensor("bvrow", [1, GC], F32, kind="ExternalInput").ap()
    d["wo"] = nc.dram_tensor("wo", [GC, DIM], F32R, kind="ExternalInput").ap()
    d["cosT"] = nc.dram_tensor("cosT", [64, S], F32R, kind="ExternalInput").ap()
    d["sinT"] = nc.dram_tensor("sinT", [64, S], F32R, kind="ExternalInput").ap()
    d["perm64"] = nc.dram_tensor("perm64", [64, 64], F32R, kind="ExternalInput").ap()
    d["ones64"] = nc.dram_tensor("ones64", [1, 64], F32R, kind="ExternalInput").ap()
    # [n_quarter, m_chunk, 128, 512]: each (n, m) tile is contiguous 256KB
    d["outT4"] = nc.dram_tensor(
        "outT4", [4, 8, 128, 512], BF16, kind="ExternalOutput"
    ).ap()

    with tile.TileContext(nc) as tc, ExitStack() as ctx:
        if reps == 1:
            _emit_body(nc, tc, ctx, d)
        else:
            def body(_iv):
                with ExitStack() as inner:
                    _emit_body(nc, tc, inner, d)

            with tc.For_i(0, reps, 1) as iv:
                body(iv)
    nc.compile()
    return nc


def shard_inputs(x, cos, sin, Wq, bq, Wk, bk, Wv, bv, Wo, bo):
    """Build the per-core input maps (host-side sharding)."""
    bf16 = mybir.dt.np(BF16)
    x = np.asarray(x, np.float32)
    cos = np.asarray(cos, np.float32).reshape(S, 64)
    sin = np.asarray(sin, np.float32).reshape(S, 64)
    sgn = np.tile(np.array([-1.0, 1.0], np.float32), 32)
    cosT = np.ascontiguousarray(cos.T)
    sinT = np.ascontiguousarray((sin * sgn).T)
    ones_cos = np.ones((64, S), np.float32)
    zero_sin = np.zeros((64, S), np.float32)
    xTs = [np.ascontiguousarray(x[b].T.astype(bf16)) for b in range(B)]
    perm64 = np.zeros((64, 64), np.float32)
    for k in range(64):
        perm64[k, k ^ 1] = 1.0

    in_maps = []
    for c in range(N_CORES):
        b, g = divmod(c, TP)
        sl = slice(GC * g, GC * (g + 1))
        m = {
            "xT": xTs[b],
            "wq": np.ascontiguousarray(np.asarray(Wq)[sl, :].T.astype(bf16)),
            "wk": np.ascontiguousarray(np.asarray(Wk)[sl, :].T.astype(bf16)),
            "wv": np.ascontiguousarray(np.asarray(Wv)[sl, :].T.astype(bf16)),
            "bq2": np.ascontiguousarray(np.asarray(bq, np.float32)[sl].reshape(2, 128).T),
            "bk2": np.ascontiguousarray(np.asarray(bk, np.float32)[sl].reshape(2, 128).T),
            "bvrow": np.asarray(bv, np.float32)[sl].reshape(1, GC).copy(),
            "wo": np.ascontiguousarray(np.asarray(Wo)[:, sl].T),
            "cosT": cosT if g == 0 else ones_cos,
            "sinT": sinT if g == 0 else zero_sin,
            "perm64": perm64,
            "ones64": np.ones((1, 64), np.float32),
        }
        in_maps.append(m)
    return in_maps


def unshard_output(results, bo):
    bo = np.asarray(bo, np.float32)
    out = np.empty((B, S, DIM), np.float32)
    for b in range(B):
        acc = np.zeros((4, 8, 128, 512), np.float32)
        for g in range(TP):
            acc += np.asarray(results[TP * b + g]["outT4"], np.float32)
        outT = acc.transpose(1, 2, 0, 3).reshape(DIM, S)
        out[b] = outT.T + bo
    return out


_NC_CACHE = {}


def get_nc(reps: int = 1, phases=(1, 2, 3)):
    key = (reps, tuple(phases))
    if key not in _NC_CACHE:
        _NC_CACHE[key] = build_nc(reps, phases)
    return _NC_CACHE[key]


def kernel(x, cos, sin, Wq, bq, Wk, bk, Wv, bv, Wo, bo, mask=None, _reps=1):
    nc = get_nc(_reps)
    in_maps = shard_inputs(x, cos, sin, Wq, bq, Wk, bk, Wv, bv, Wo, bo)
    res = run_bass_kernel_spmd(nc, in_maps, list(range(N_CORES)))
    return unshard_output(res.results, bo)


# revision 38
# speedup vs baseline: 1.0148x; 1.0148x over previous
"""DiT attention kernel for 8 Trainium2 NeuronCores.

Sharding: tensor-parallel over head groups (4 groups of 4 heads) x
data-parallel over batch (2), giving 8 cores. Each core computes, for its
batch b and head group g:
    QT/KT = (x[b] @ W{q,k}[g].T + b)  in [head_dim, seq] layout
    partial rotary on global head 0 (cores with g==0; others get cos=1/sin=0)
    V in [seq, head_dim] layout (computed transposed then PE-transposed)
    S^T = K Q^T per head, P^T = exp(S^T/8), O^T = V_aug^T P^T (ones column
    in V gives softmax denominators for free), normalize, then the
    row-parallel Wo partial product out^T = Wo[g].T @ O^T.
The host sums the 4 partial out^T per batch, transposes, and adds bo.

Matmuls run in float32r (single-pass PE mode, ~1.5e-4 rel err).
"""

import sys

if "/opt/trn_rl_repo" not in sys.path:
    sys.path.insert(0, "/opt/trn_rl_repo")

from contextlib import ExitStack

import numpy as np

import concourse.bass as bass  # noqa: F401  (bass must import before bacc)
import concourse.mybir as mybir
import concourse.tile as tile
from concourse import bacc
from concourse.bass_utils import run_bass_kernel_spmd
from concourse.masks import make_identity

F32 = mybir.dt.float32
F32R = mybir.dt.float32r

B, S, DIM, HEADS, HEAD_DIM = 2, 2048, 1024, 16, 64
N_CORES = 8
TP = 4                      # head groups
GH = HEADS // TP            # heads per core (4)
GC = GH * HEAD_DIM          # cols per core slice (256)
EXP_FN = mybir.ActivationFunctionType.Exp


def _emit_body(nc, tc, ctx, d, phases=(1, 2, 3), inplace_rope=True):
    """Emit one pass of the kernel body. d = dict of dram APs."""
    consts = ctx.enter_context(tc.tile_pool(name="consts", bufs=1))

    ones4 = consts.tile([128, 4], F32, name="ones4", tag="ones4")
    nc.vector.memset(ones4[:], 1.0)

    # Persistent activations
    qt = [consts.tile([128, S], F32R, name=f"qt{i}", tag=f"qt{i}") for i in range(2)]
    kt = [consts.tile([128, S], F32R, name=f"kt{i}", tag=f"kt{i}") for i in range(2)]
    if not inplace_rope:
        qtr = consts.tile([64, S], F32R, name="qtr", tag="qtr")
        ktr = consts.tile([64, S], F32R, name="ktr", tag="ktr")
    # V in natural layout + ones column per head: head h at cols [65h, 65h+64],
    # col 65h+64 is 1.0 (gives softmax sums as output row 64 of the PV matmul).
    vstore = [consts.tile([128, 65 * GH], F32R, name=f"vs{i}", tag=f"vs{i}") for i in range(16)]
    otst = [consts.tile([128, S], F32R, name=f"ot{i}", tag=f"ot{i}") for i in range(2)]

    bq_sb = consts.tile([128, 2], F32, name="bq", tag="bq")
    bk_sb = consts.tile([128, 2], F32, name="bk", tag="bk")
    bvrep = consts.tile([128, GC], F32, name="bvrep", tag="bvrep")
    nc.sync.dma_start(out=bq_sb[:], in_=d["bq2"][:, :])
    nc.sync.dma_start(out=bk_sb[:], in_=d["bk2"][:, :])
    nc.sync.dma_start(out=bvrep[:], in_=d["bvrow"][:].to_broadcast([128, GC]))

    # ---------------- Phase 1: QKV projections (+rope) ---------------------
    with tc.tile_pool(name="xw", bufs=1) as xw:
        # interleave the wq-chunk and xt-chunk loads so the first Q matmuls
        # can start as soon as the first ~1MB lands
        with tc.tile_pool(name="wstream", bufs=9) as wsp:
            wq = [wsp.tile([128, GC], F32R, name="w", tag="w") for _ in range(8)]
            xt = [xw.tile([128, S], F32R, name=f"xt{k}", tag=f"xt{k}") for k in range(8)]
            for k in range(8):
                nc.sync.dma_start(out=wq[k][:], in_=d["wq"][128 * k : 128 * (k + 1), :])
                nc.sync.dma_start(out=xt[k][:], in_=d["xT"][128 * k : 128 * (k + 1), :])
            cos_sb = xw.tile([64, S], F32R, name="cos", tag="cos")
            sin_sb = xw.tile([64, S], F32R, name="sin", tag="sin")
            nc.sync.dma_start(out=cos_sb[:], in_=d["cosT"][:, :])
            nc.sync.dma_start(out=sin_sb[:], in_=d["sinT"][:, :])

            with tc.tile_pool(name="pq", bufs=2, space="PSUM") as pq:
                for nm, wd, w, bias_sb, dest in (
                    ("q", d["wq"], wq, bq_sb, qt),
                    ("k", d["wk"], None, bk_sb, kt),
                ):
                    if w is None:
                        w = [wsp.tile([128, GC], F32R, name="w", tag="w") for _ in range(8)]
                        for k in range(8):
                            nc.sync.dma_start(
                                out=w[k][:], in_=wd[128 * k : 128 * (k + 1), :]
                            )
                    for m in range(2):
                        ps = pq.tile([128, S], F32, name="pqkv", tag="pqkv")
                        for n in range(4):
                            for k in range(8):
                                nc.tensor.matmul(
                                    ps[:, 512 * n : 512 * (n + 1)],
                                    lhsT=w[k][:, 128 * m : 128 * (m + 1)],
                                    rhs=xt[k][:, 512 * n : 512 * (n + 1)],
                                    start=(k == 0),
                                    stop=(k == 7),
                                )
                        nc.vector.tensor_scalar_add(
                            out=dest[m][:], in0=ps[:], scalar1=bias_sb[:, m : m + 1]
                        )
                # V directly in natural [seq, vdim] layout (no PE transpose):
                # lhsT = x^T chunks, rhs = wv; bias broadcast along partitions
                wv = [wsp.tile([128, GC], F32R, name="w", tag="w") for _ in range(8)]
                for k in range(8):
                    nc.sync.dma_start(
                        out=wv[k][:], in_=d["wv"][128 * k : 128 * (k + 1), :]
                    )
                for blk in range(16):
                    nc.vector.tensor_copy(vstore[blk][:, 64 : 65 * GH : 65], ones4[:])
                for sc in range(16):
                    psv = pq.tile([128, GC], F32, name="pqkv", tag="pqkv")
                    for k in range(8):
                        nc.tensor.matmul(
                            psv[:],
                            lhsT=xt[k][:, 128 * sc : 128 * (sc + 1)],
                            rhs=wv[k][:, :],
                            start=(k == 0),
                            stop=(k == 7),
                        )
                    dst = (
                        vstore[sc][:, 0 : 65 * GH]
                        .rearrange("p (h c) -> p h c", h=GH)[:, :, 0:64]
                    )
                    nc.vector.tensor_add(
                        dst,
                        psv[:].rearrange("p (h c) -> p h c", h=GH),
                        bvrep[:].rearrange("p (h c) -> p h c", h=GH),
                    )

            # rotary on local head 0 (dims 0:64 of qt[0]/kt[0]); other
            # cores receive cos=1/sin=0 so this is an identity there.
            with tc.tile_pool(name="rope", bufs=1) as rp:
                for src, j in ((qt[0], 0), (kt[0], 1)):
                    dst = src[0:64, :] if inplace_rope else (qtr, ktr)[j][:]
                    sw = rp.tile([64, S], F32R, name=f"sw{j}", tag="ropetmp", bufs=2)
                    nc.sync.dma_start(out=sw[0:64:2, :], in_=src[1:64:2, :])
                    nc.sync.dma_start(out=sw[1:64:2, :], in_=src[0:64:2, :])
                    t1 = rp.tile([64, S], F32R, name=f"t1{j}", tag="ropetmp", bufs=2)
                    nc.vector.tensor_mul(t1[:], sw[:], sin_sb[:])
                    nc.vector.tensor_mul(dst, src[0:64, :], cos_sb[:])
                    nc.vector.tensor_add(dst, dst, t1[:])

    # ---------------- Phase 2: attention -----------------------------------
    if 2 in phases:
        with tc.tile_pool(name="ptp", bufs=6) as ptp, tc.tile_pool(
            name="stp", bufs=3, space="PSUM"
        ) as stp, tc.tile_pool(name="otp", bufs=2, space="PSUM") as otp, tc.tile_pool(
            name="nrm", bufs=3
        ) as nrm, tc.tile_pool(name="dscr", bufs=8, space="DRAM") as dscr:
            for p in range(2):  # head pair
                for qq in range(4):  # query quarter (512 wide)
                    ots = [otp.tile([128, 512], F32, name="otps", tag="otps") for _ in range(2)]
                    # sweep A: scores + exp for all 16 key blocks (PT fully
                    # materialized in SBUF); sweep B: the 32 PV matmuls.
                    # Keeps PE in one tile mode per sweep and lets the next
                    # quarter's exps overlap this quarter's PV matmuls.
                    pts = []
                    for blk in range(16):
                        st = stp.tile([128, 1024], F32, name="st", tag="st")
                        for hh in range(2):
                            rope = p == 0 and hh == 0 and not inplace_rope
                            k_ap = (
                                ktr[:, 128 * blk : 128 * (blk + 1)]
                                if rope
                                else kt[p][
                                    64 * hh : 64 * (hh + 1), 128 * blk : 128 * (blk + 1)
                                ]
                            )
                            q_ap = (
                                qtr[:, 512 * qq : 512 * (qq + 1)]
                                if rope
                                else qt[p][
                                    64 * hh : 64 * (hh + 1), 512 * qq : 512 * (qq + 1)
                                ]
                            )
                            nc.tensor.matmul(
                                st[:, 512 * hh : 512 * (hh + 1)],
                                lhsT=k_ap,
                                rhs=q_ap,
                                start=True,
                                stop=True,
                            )
                        pt = ptp.tile([128, 1024], F32R, name="pt", tag="pt", bufs=18)
                        nc.scalar.activation(pt[:], st[:], EXP_FN, scale=0.125)
                        pts.append(pt)
                    for blk in range(16):
                        for hh in range(2):
                            h = 2 * p + hh
                            nc.tensor.matmul(
                                ots[hh][0:65, :],
                                lhsT=vstore[blk][:, 65 * h : 65 * h + 65],
                                rhs=pts[blk][:, 512 * hh : 512 * (hh + 1)],
                                start=(blk == 0),
                                stop=(blk == 15),
                            )
                    # evict the un-normalized O^T + denominators to SBUF right
                    # away so the PSUM banks recycle without waiting on the
                    # normalization chain (which has two DRAM round-trips).
                    for hh in range(2):
                        ot_un = nrm.tile([128, 512], F32, name="ot_un", tag="ot_un", bufs=4)
                        nc.vector.tensor_copy(ot_un[0:65, :], ots[hh][0:65, :])
                        scr1 = dscr.tile([1, 512], F32, name="scr1", tag="scr1")
                        nc.sync.dma_start(out=scr1[:], in_=ot_un[64:65, :])
                        rst = nrm.tile([128, 4], F32, name="rst", tag="rst")
                        nc.sync.dma_start(
                            out=rst[:],
                            in_=scr1[:].rearrange("o (p f) -> (o p) f", p=128),
                        )
                        nc.vector.reciprocal(rst[:], rst[:])
                        scr2 = dscr.tile([1, 512], F32, name="scr2", tag="scr2")
                        nc.sync.dma_start(
                            out=scr2[:].rearrange("o (p f) -> (o p) f", p=128),
                            in_=rst[:],
                        )
                        bc = nrm.tile([64, 512], F32, name="bc", tag="bc")
                        nc.sync.dma_start(out=bc[:], in_=scr2[:].to_broadcast([64, 512]))
                        if hh == 0:
                            nc.vector.tensor_mul(
                                otst[p][0:64, 512 * qq : 512 * (qq + 1)],
                                ot_un[0:64, :],
                                bc[:].bitcast(F32R),
                            )
                        else:
                            # DVE cannot write partition base 64 from base-0
                            # inputs; go through a temp tile + sbuf->sbuf DMA.
                            tmp = nrm.tile([64, 512], F32R, name="tmp", tag="tmp")
                            nc.vector.tensor_mul(tmp[:], ot_un[0:64, :], bc[:].bitcast(F32R))
                            nc.sync.dma_start(
                                out=otst[p][64:128, 512 * qq : 512 * (qq + 1)],
                                in_=tmp[:],
                            )

    # ---------------- Phase 3: output projection (row-parallel partial) ----
    if 3 in phases:
        with tc.tile_pool(name="wop", bufs=1) as wop, tc.tile_pool(
            name="pw", bufs=4, space="PSUM"
        ) as pw:
            wo_sb = [wop.tile([128, DIM], F32R, name=f"wo{k}", tag=f"wo{k}") for k in range(2)]
            for k in range(2):
                nc.sync.dma_start(
                    out=wo_sb[k][:], in_=d["wo"][128 * k : 128 * (k + 1), :]
                )
            for m in range(8):
                for n in range(4):
                    ps = pw.tile([128, 512], F32, name="pwo", tag="pwo")
                    for k in range(2):
                        nc.tensor.matmul(
                            ps[:],
                            lhsT=wo_sb[k][:, 128 * m : 128 * (m + 1)],
                            rhs=otst[k][:, 512 * n : 512 * (n + 1)],
                            start=(k == 0),
                            stop=(k == 1),
                        )
                    ob = wop.tile([128, 512], F32, name="ob", tag="ob", bufs=4)
                    nc.vector.tensor_copy(ob[:], ps[:])
                    nc.sync.dma_start(out=d["outT4"][m, n], in_=ob[:])


def build_nc(reps: int = 1, phases=(1, 2, 3)):
    nc = bacc.Bacc(
        "TRN2", target_bir_lowering=False, debug=False, num_devices=N_CORES
    )
    d = {}
    d["xT"] = nc.dram_tensor("xT", [DIM, S], F32R, kind="ExternalInput").ap()
    for nm in ("wq", "wk", "wv"):
        d[nm] = nc.dram_tensor(nm, [DIM, GC], F32R, kind="ExternalInput").ap()
    for nm in ("bq2", "bk2"):
        d[nm] = nc.dram_tensor(nm, [128, 2], F32, kind="ExternalInput").ap()
    d["bvrow"] = nc.dram_tensor("bvrow", [1, GC], F32, kind="ExternalInput").ap()
    d["wo"] = nc.dram_tensor("wo", [GC, DIM], F32R, kind="ExternalInput").ap()
    d["cosT"] = nc.dram_tensor("cosT", [64, S], F32R, kind="ExternalInput").ap()
    d["sinT"] = nc.dram_tensor("sinT", [64, S], F32R, kind="ExternalInput").ap()
    # each (m, n) output tile is a contiguous 256KB block
    d["outT4"] = nc.dram_tensor(
        "outT4", [8, 4, 128, 512], F32, kind="ExternalOutput"
    ).ap()

    inplace_rope = reps == 1
    with tile.TileContext(nc) as tc, ExitStack() as ctx:
        if reps == 1:
            _emit_body(nc, tc, ctx, d, phases, inplace_rope)
        else:
            def body(_iv):
                with ExitStack() as inner:
                    _emit_body(nc, tc, inner, d, phases, inplace_rope)

            with tc.For_i(0, reps, 1) as iv:
                body(iv)
    nc.compile()
    return nc


def shard_inputs(x, cos, sin, Wq, bq, Wk, bk, Wv, bv, Wo, bo):
    """Build the per-core input maps (host-side sharding)."""
    x = np.asarray(x, np.float32)
    cos = np.asarray(cos, np.float32).reshape(S, 64)
    sin = np.asarray(sin, np.float32).reshape(S, 64)
    sgn = np.tile(np.array([-1.0, 1.0], np.float32), 32)
    cosT = np.ascontiguousarray(cos.T)
    sinT = np.ascontiguousarray((sin * sgn).T)
    ones_cos = np.ones((64, S), np.float32)
    zero_sin = np.zeros((64, S), np.float32)
    xTs = [np.ascontiguousarray(x[b].T) for b in range(B)]

    in_maps = []
    for c in range(N_CORES):
        b, g = divmod(c, TP)
        sl = slice(GC * g, GC * (g + 1))
        m = {
            "xT": xTs[b],
            "wq": np.ascontiguousarray(np.asarray(Wq)[sl, :].T),
            "wk": np.ascontiguousarray(np.asarray(Wk)[sl, :].T),
            "wv": np.ascontiguousarray(np.asarray(Wv)[sl, :].T),
            "bq2": np.ascontiguousarray(np.asarray(bq, np.float32)[sl].reshape(2, 128).T),
            "bk2": np.ascontiguousarray(np.asarray(bk, np.float32)[sl].reshape(2, 128).T),
            "bvrow": np.asarray(bv, np.float32)[sl].reshape(1, GC).copy(),
            "wo": np.ascontiguousarray(np.asarray(Wo)[:, sl].T),
            "cosT": cosT if g == 0 else ones_cos,
            "sinT": sinT if g == 0 else zero_sin,
        }
        in_maps.append(m)
    return in_maps


def unshard_output(results, bo):
    bo = np.asarray(bo, np.float32)
    out = np.empty((B, S, DIM), np.float32)
    for b in range(B):
        acc = np.zeros((8, 4, 128, 512), np.float32)
        for g in range(TP):
            acc += results[TP * b + g]["outT4"]
        outT = acc.transpose(0, 2, 1, 3).reshape(DIM, S)
        out[b] = outT.T + bo
    return out


_NC_CACHE = {}


def get_nc(reps: int = 1, phases=(1, 2, 3)):
    key = (reps, tuple(phases))
    if key not in _NC_CACHE:
        _NC_CACHE[key] = build_nc(reps, phases)
    return _NC_CACHE[key]


def kernel(x, cos, sin, Wq, bq, Wk, bk, Wv, bv, Wo, bo, mask=None, _reps=1):
    nc = get_nc(_reps)
    in_maps = shard_inputs(x, cos, sin, Wq, bq, Wk, bk, Wv, bv, Wo, bo)
    res = run_bass_kernel_spmd(nc, in_maps, list(range(N_CORES)))
    return unshard_output(res.results, bo)

